# revision 15
# baseline (speedup 1.0000x reference)
"""Causal self-attention (B=4, T=2048, E=1024, H=16) on 8 trn2 NeuronCores.

Sharding: core c -> (batch b = c // 2, head-group hg = c % 2); each core owns
one batch element and 8 of the 16 heads (data parallel on B, tensor parallel
on heads).  No cross-core communication.

v2 design (all-bf16 attention, flipped PV):
  - QKV projection in f32r (1 cyc/row at 512 moving); q/k bias-add moved to
    DVE (tensor_scalar_add, psum f32 -> sbuf bf16), freeing ScalarE for exp.
  - QK: bf16 row-tiled head pairs (2 heads per 512-cycle pass), causal
    trim per j-tile (moving width 512-128r on diagonal tiles).
  - exp on ScalarE per j-tile (st psum [128, 2head, 512] -> pt bf16), width
    trimmed like QK; causal mask via gpsimd affine_select on the 128-wide
    diagonal band of pt only.
  - PV flipped: pt is the STATIONARY operand ([j, i-tile] 128 cols), v the
    moving one ([j, d+ones] 65 cols) -> out yt [i-part, 65] costs 65 cycles
    per (head, j-tile, i-tile) vs 512 for the [d, i] orientation.  The
    softmax denominator (ones column of v) lands per-partition, so
    normalization is a [128,8] reciprocal + per-partition-scalar multiplies
    on DVE - no cross-partition broadcast, no DRAM bounce.
  - Output written as y[t, c]; host concatenates without transposing.
"""

import sys

sys.path.insert(0, "/opt/trn_rl_repo")

import numpy as np

N_CORES = 8
B, T, E = 4, 2048, 1024
H, D = 16, 64
C = E                 # q/k/v channel count (4th qkv chunk unused)
HPC = H // 2          # heads per core
CC = HPC * D          # per-core channels = 512
ES = E // 128         # 8 e-tiles (contraction)
TB = T // 512         # 4 t/i blocks of 512
NJ = T // 128         # 16 j-tiles of 128
PAIRS = HPC // 2      # 4 head pairs per core

_cache = {}


def _build_nc():
    import concourse.mybir as mybir
    import concourse.tile as tile
    from concourse import bacc

    f32 = mybir.dt.float32
    f32r = mybir.dt.float32r
    bf16 = mybir.dt.bfloat16
    Act = mybir.ActivationFunctionType
    is_ge = mybir.AluOpType.is_ge

    nc = bacc.Bacc("TRN2", target_bir_lowering=False, debug=False)

    xT = nc.dram_tensor("xT", [E, T], f32r, kind="ExternalInput").ap()
    w_qk = nc.dram_tensor("w_qk", [E, 2 * CC], f32r, kind="ExternalInput").ap()
    w_v = nc.dram_tensor("w_v", [E, CC], f32r, kind="ExternalInput").ap()
    b_qk = nc.dram_tensor("b_qk", [128, 8], f32, kind="ExternalInput").ap()
    b_v = nc.dram_tensor("b_v", [1, CC], f32r, kind="ExternalInput").ap()
    ones_d = nc.dram_tensor("ones_d", [1, 128], f32r, kind="ExternalInput").ap()
    y_out = nc.dram_tensor("y_out", [T, CC], f32, kind="ExternalOutput").ap()

    with tile.TileContext(nc) as tc:
        with (
            tc.tile_pool(name="persist", bufs=1) as pp,
            tc.tile_pool(name="psum", bufs=1, space="PSUM") as psp,
            tc.tile_pool(name="xpool", bufs=2) as xp,
            tc.tile_pool(name="ptpool", bufs=5) as ptp,
            tc.tile_pool(name="opool", bufs=1) as op,
        ):
            # ---- persistent SBUF state ----
            qk_sb = [pp.tile([128, T], bf16, name=f"qk{ct}") for ct in range(8)]
            # v plus a ones column per head: [t-part, head, j-tile, 65]
            v1_sb = pp.tile([128, HPC, NJ, D + 1], bf16, name="v1")
            bqk_sb = pp.tile([128, 8], f32, name="bqk")
            bv_sb = pp.tile([1, CC], f32r, name="bv")
            ones_sb = pp.tile([1, 128], f32r, name="ones")
            wqk_t = []
            wv_t = []

            # softmax-denominator ones column (written once; v copies fill 0:D)
            nc.gpsimd.memset(v1_sb[:, :, :, D : D + 1], 1.0)

            xs_tb = {}

            def load_x(tb):
                tsl = slice(tb * 512, (tb + 1) * 512)
                xs = []
                for e in range(ES):
                    xe = xp.tile([128, 512], f32r, tag=f"x{e}", bufs=2,
                                 name=f"x{e}_{tb}")
                    nc.sync.dma_start(out=xe, in_=xT[e * 128 : (e + 1) * 128, tsl])
                    xs.append(xe)
                xs_tb[tb] = xs

            # small constants first, then x/w interleaved per e-tile so the
            # first matmul accumulation group can finish as early as possible
            nc.sync.dma_start(out=bqk_sb, in_=b_qk)
            nc.sync.dma_start(out=bv_sb, in_=b_v)
            nc.sync.dma_start(out=ones_sb, in_=ones_d)
            tsl0 = slice(0, 512)
            xs0 = []
            for e in range(ES):
                xe = xp.tile([128, 512], f32r, tag=f"x{e}", bufs=2,
                             name=f"x{e}_0")
                nc.sync.dma_start(out=xe, in_=xT[e * 128 : (e + 1) * 128, tsl0])
                xs0.append(xe)
                wv = pp.tile([128, CC], f32r, name=f"wv{e}")
                nc.sync.dma_start(out=wv, in_=w_v[e * 128 : (e + 1) * 128, :])
                wv_t.append(wv)
            xs_tb[0] = xs0
            # (host packs w_qk cols pr-major: pr*256+[0:128]=q, +[128:256]=k)
            for e in range(ES):
                wqk = pp.tile([128, 2 * CC], f32r, name=f"wqk{e}")
                nc.sync.dma_start(out=wqk, in_=w_qk[e * 128 : (e + 1) * 128, :])
                wqk_t.append(wqk)

            def qkv_group_qk(tb, ct, lo=0, hi=ES, cell=None):
                """Emit e-tiles [lo, hi) of the ct projection group; the
                last chunk appends the DVE bias-add.  cell carries the psum
                tile between chunks so groups can be woven in small pieces
                that don't block QK matmuls on the in-order PE queue."""
                tsl = slice(tb * 512, (tb + 1) * 512)
                xs = xs_tb[tb]
                co = (ct % 4) * 256 + (128 if ct >= 4 else 0)
                if cell is None:
                    cell = {}
                if lo == 0:
                    cell["ps"] = psp.tile([128, 512], f32, tag="qp", bufs=2,
                                          name=f"psqk{ct}_{tb}")
                ps = cell["ps"]
                for e in range(lo, hi):
                    nc.tensor.matmul(
                        ps,
                        wqk_t[e][:, co : co + 128],
                        xs[e],
                        start=(e == 0),
                        stop=(e == ES - 1),
                        skip_group_check=True,
                    )
                if hi == ES:
                    # bias add on DVE (psum f32 + [128,1] bias -> sbuf bf16)
                    nc.vector.tensor_scalar_add(
                        qk_sb[ct][:, tsl], ps, bqk_sb[:, ct : ct + 1])

            def qkv_group_v(tb, k4, lo=0, hi=ES, cell=None):
                xs = xs_tb[tb]
                tt = tb * 4 + k4
                if cell is None:
                    cell = {}
                if lo == 0:
                    cell["ps"] = psp.tile([128, 512], f32, tag="qp", bufs=2,
                                          name=f"psv{tt}")
                    nc.tensor.matmul(
                        cell["ps"], ones_sb, bv_sb,
                        start=True, stop=False, skip_group_check=True,
                    )
                psv = cell["ps"]
                for e in range(lo, hi):
                    nc.tensor.matmul(
                        psv,
                        xs[e][:, k4 * 128 : (k4 + 1) * 128],
                        wv_t[e],
                        start=False,
                        stop=(e == ES - 1),
                        skip_group_check=True,
                    )
                if hi == ES:
                    nc.vector.tensor_copy(
                        v1_sb[:, :, tt, 0:D],
                        psv.rearrange("p (h d) -> p h d", d=D),
                    )

            def qkv_steps(tb):
                """Chunked emission steps for t-block tb's full projection."""
                steps = []
                for ct in range(8):
                    cell = {}
                    for lo, hi in ((0, 3), (3, 6), (6, 8)):
                        steps.append(
                            lambda c=ct, l=lo, h=hi, ce=cell:
                            qkv_group_qk(tb, c, l, h, ce))
                for g in range(4):
                    cell = {}
                    for lo, hi in ((0, 3), (3, 6), (6, 8)):
                        steps.append(
                            lambda k=g, l=lo, h=hi, ce=cell:
                            qkv_group_v(tb, k, l, h, ce))
                return steps

            def attn_block(I, nxt=(), pre=None):
                nj = 4 * I + 4  # causal j-tiles for this i-block
                yts = {}
                pts = {}

                def qk_exp(pr, J):
                    r = J - 4 * I
                    ws = 128 * r if r > 0 else 0  # causal trim offset
                    qt = qk_sb[pr]
                    kt = qk_sb[4 + pr]
                    jsl = slice(J * 128, (J + 1) * 128)
                    iwl = slice(I * 512 + ws, (I + 1) * 512)
                    st = psp.tile([128, 2, 512], f32, tag="st", bufs=2,
                                  name=f"st{pr}_{I}_{J}")
                    # QK row-tile pair: head A rows 0-63, head B 64-127
                    nc.tensor.matmul(
                        st[:, 0, ws:], kt[0:64, jsl], qt[0:64, iwl],
                        tile_position=(0, 0),
                    )
                    nc.tensor.matmul(
                        st[:, 1, ws:], kt[64:128, jsl], qt[64:128, iwl],
                        tile_position=(64, 0),
                    )
                    pt = ptp.tile([128, 2, 512], bf16, tag="pt",
                                  name=f"pt{pr}_{I}_{J}")
                    nc.scalar.activation(pt[:, :, ws:], st[:, :, ws:],
                                         Act.Exp, scale=0.125)
                    if r >= 0:
                        # causal mask on the 128-wide diagonal band only:
                        # keep where i_band >= j (within-tile coords)
                        nc.gpsimd.affine_select(
                            out=pt[:, :, 128 * r : 128 * (r + 1)],
                            in_=pt[:, :, 128 * r : 128 * (r + 1)],
                            compare_op=is_ge,
                            fill=0.0,
                            base=0,
                            pattern=[[0, 2], [1, 128]],
                            channel_multiplier=-1,
                        )
                    pts[(pr, J)] = pt

                def pv(pr, J):
                    pt = pts.pop((pr, J))
                    ytA, ytB = yts[pr]
                    r = J - 4 * I
                    for h, yt in ((0, ytA), (1, ytB)):
                        for it in range(4):
                            if r > it:
                                continue  # i-tile fully masked for this j
                            # PSUM zeroing is bank-granular: only the FIRST
                            # region of each bank sets start=True; the other
                            # it regions' first writes land on pending-zero
                            # bytes and get a zeroed accumulation base.
                            nc.tensor.matmul(
                                yt[:, it, 0 : D + 1],
                                pt[:, h, it * 128 : (it + 1) * 128],
                                v1_sb[:, 2 * pr + h, J, :],
                                start=(J == 0 and it == 0),
                                stop=(J == 4 * I + it),
                                skip_group_check=True,
                            )

                def out_stage(pr):
                    ytA, ytB = yts.pop(pr)
                    rec = op.tile([128, 2, 4], f32, tag="rec", bufs=2,
                                  name=f"rec{pr}_{I}")
                    ys = op.tile([128, 2, 4, D], f32, tag="ys", bufs=2,
                                 name=f"ys{pr}_{I}")
                    # denominators live in column 64 of each (h, it) slot
                    nc.vector.reciprocal(rec[:, 0, :], ytA[:, :, D])
                    nc.vector.reciprocal(rec[:, 1, :], ytB[:, :, D])
                    for h, yt in ((0, ytA), (1, ytB)):
                        for it in range(4):
                            nc.vector.tensor_scalar_mul(
                                ys[:, h, it, :], yt[:, it, 0:D],
                                rec[:, h, it : it + 1])
                    # ys [i-part, h, it, d] -> y_out rows I*512+it*128+i,
                    # cols (2pr+h)*64+d  (one DMA per head: 3 free dims max)
                    import concourse.bass as bass

                    for h in (0, 1):
                        out_ap = bass.AP(
                            tensor=y_out.tensor,
                            offset=(I * 512) * CC + (2 * pr + h) * D,
                            ap=[[CC, 128], [128 * CC, 4], [1, D]],
                        )
                        nc.sync.dma_start(
                            out=out_ap,
                            in_=ys[:, h, :, :],
                        )

                def alloc_yt(pr):
                    # padded to 4x128 = one full 2KB PSUM bank per head so
                    # the bank-granular start=True zeroing touches no other
                    # tile; regions are [it, 0:65] (64 d cols + denominator)
                    yts[pr] = (
                        psp.tile([128, 4, 128], f32, tag="ytA", bufs=1,
                                 name=f"ytA{pr}_{I}"),
                        psp.tile([128, 4, 128], f32, tag="ytB", bufs=1,
                                 name=f"ytB{pr}_{I}"),
                    )

                items = [(pr, J) for pr in range(PAIRS) for J in range(nj)]
                nxt = list(nxt)
                nsteps = len(nxt)
                popped = 0
                emitted = 0

                def emit_qk(k):
                    pr, J = items[k]
                    if J == 0:
                        if pre:
                            for fn in pre[pr]:
                                fn()
                        alloc_yt(pr)
                    qk_exp(pr, J)

                for k in range(len(items)):
                    while emitted < min(k + 3, len(items)):
                        emit_qk(emitted)
                        emitted += 1
                    pr, J = items[k]
                    pv(pr, J)
                    if J == nj - 1:
                        out_stage(pr)
                    # weave next t-block's QKV in small chunks so a long
                    # projection burst never delays the next QK (which
                    # would starve ScalarE's exp pipeline)
                    want = (k + 1) * nsteps // len(items)
                    while popped < want:
                        nxt[popped]()
                        popped += 1
                for fn in nxt[popped:]:
                    fn()

            # schedule: v(0) groups first, then per-pair q/k groups woven in
            # front of each pair's attention (pre), then per t-block
            # attention with the next block's QKV woven in.
            for g in range(4):
                qkv_group_v(0, g)
            pre0 = {
                pr: [
                    (lambda c=pr: qkv_group_qk(0, c)),
                    (lambda c=4 + pr: qkv_group_qk(0, c)),
                ]
                for pr in range(PAIRS)
            }
            for I in range(TB):
                nxt = []
                if I + 1 < TB:
                    load_x(I + 1)
                    nxt = qkv_steps(I + 1)
                attn_block(I, nxt, pre=(pre0 if I == 0 else None))
    nc.compile()
    return nc


def get_nc():
    if "nc" not in _cache:
        _cache["nc"] = _build_nc()
    return _cache["nc"]


def shard_inputs(x, w_attn, b_attn):
    """Full inputs -> per-core input maps (host-side slicing/transposition)."""
    x = np.asarray(x, dtype=np.float32)
    w = np.asarray(w_attn, dtype=np.float32)
    bb = np.asarray(b_attn, dtype=np.float32)
    in_maps = []
    for core in range(N_CORES):
        b, hg = core // 2, core % 2
        r0 = hg * CC  # first q row for this head group
        # head-pair-major column packing: pr*256+[0:128]=q(pr), +[128:256]=k(pr)
        wq = w[r0 : r0 + CC, :]
        wk = w[C + r0 : C + r0 + CC, :]
        w_qk = np.ascontiguousarray(
            np.concatenate(
                sum(
                    (
                        [wq[pr * 128 : (pr + 1) * 128], wk[pr * 128 : (pr + 1) * 128]]
                        for pr in range(PAIRS)
                    ),
                    [],
                ),
                axis=0,
            ).T
        )
        w_v = np.ascontiguousarray(w[2 * C + r0 : 2 * C + r0 + CC, :].T)
        b_qk = np.stack(
            [bb[r0 + ct * 128 : r0 + (ct + 1) * 128] for ct in range(4)]
            + [bb[C + r0 + ct * 128 : C + r0 + (ct + 1) * 128] for ct in range(4)],
            axis=1,
        ).astype(np.float32)
        b_v = bb[2 * C + r0 : 2 * C + r0 + CC].reshape(1, CC).astype(np.float32)
        in_maps.append(
            {
                "xT": np.ascontiguousarray(x[b].T),
                "w_qk": w_qk,
                "w_v": w_v,
                "b_qk": np.ascontiguousarray(b_qk),
                "b_v": np.ascontiguousarray(b_v),
                "ones_d": np.ones((1, 128), dtype=np.float32),
            }
        )
    return in_maps


def run(in_maps, trace=False, **kw):
    from concourse import bass_utils

    nc = get_nc()
    return bass_utils.run_bass_kernel_spmd(
        nc, in_maps, core_ids=list(range(N_CORES)), trace=trace, **kw
    )


def gather_output(results):
    y = np.empty((B, T, E), dtype=np.float32)
    for core in range(N_CORES):
        b, hg = core // 2, core % 2
        y[b, :, hg * CC : (hg + 1) * CC] = results[core]["y_out"]
    return y


def kernel(x, w_attn, b_attn):
    in_maps = shard_inputs(x, w_attn, b_attn)
    res = run(in_maps, trace=False)
    return gather_output(res.results)


# revision 33
# speedup vs baseline: 1.0150x; 1.0150x over previous
"""Causal self-attention (B=4, T=2048, E=1024, H=16) on 8 trn2 NeuronCores.

Sharding: core c -> (batch b = c // 2, head-group hg = c % 2); each core owns
one batch element and 8 of the 16 heads (data parallel on B, tensor parallel
on heads).  No cross-core communication.

v2 design (all-bf16 attention, flipped PV):
  - QKV projection in f32r (1 cyc/row at 512 moving); q/k bias-add moved to
    DVE (tensor_scalar_add, psum f32 -> sbuf bf16), freeing ScalarE for exp.
  - QK: bf16 row-tiled head pairs (2 heads per 512-cycle pass), causal
    trim per j-tile (moving width 512-128r on diagonal tiles).
  - exp on ScalarE per j-tile (st psum [128, 2head, 512] -> pt bf16), width
    trimmed like QK; causal mask via gpsimd affine_select on the 128-wide
    diagonal band of pt only.
  - PV flipped: pt is the STATIONARY operand ([j, i-tile] 128 cols), v the
    moving one ([j, d+ones] 65 cols) -> out yt [i-part, 65] costs 65 cycles
    per (head, j-tile, i-tile) vs 512 for the [d, i] orientation.  The
    softmax denominator (ones column of v) lands per-partition, so
    normalization is a [128,8] reciprocal + per-partition-scalar multiplies
    on DVE - no cross-partition broadcast, no DRAM bounce.
  - Output written as y[t, c]; host concatenates without transposing.
"""

import sys

sys.path.insert(0, "/opt/trn_rl_repo")

import numpy as np

N_CORES = 8
B, T, E = 4, 2048, 1024
H, D = 16, 64
C = E                 # q/k/v channel count (4th qkv chunk unused)
HPC = H // 2          # heads per core
CC = HPC * D          # per-core channels = 512
ES = E // 128         # 8 e-tiles (contraction)
TB = T // 512         # 4 t/i blocks of 512
NJ = T // 128         # 16 j-tiles of 128
PAIRS = HPC // 2      # 4 head pairs per core

_cache = {}


def _build_nc():
    import concourse.mybir as mybir
    import concourse.tile as tile
    from concourse import bacc

    f32 = mybir.dt.float32
    f32r = mybir.dt.float32r
    bf16 = mybir.dt.bfloat16
    Act = mybir.ActivationFunctionType
    is_ge = mybir.AluOpType.is_ge

    nc = bacc.Bacc("TRN2", target_bir_lowering=False, debug=False)

    xT = nc.dram_tensor("xT", [E, T], f32r, kind="ExternalInput").ap()
    w_qk = nc.dram_tensor("w_qk", [E, 2 * CC], f32r, kind="ExternalInput").ap()
    w_v = nc.dram_tensor("w_v", [E, CC], f32r, kind="ExternalInput").ap()
    b_qk = nc.dram_tensor("b_qk", [128, 8], f32, kind="ExternalInput").ap()
    b_v = nc.dram_tensor("b_v", [1, CC], f32r, kind="ExternalInput").ap()
    ones_d = nc.dram_tensor("ones_d", [1, 128], f32r, kind="ExternalInput").ap()
    # flat [(I, it, p), (head, d)] == [T, CC] row-major: row I*512+it*128+p
    # is exactly t, cols are c = head*64+d.  Kept 2-D: the PJRT lowering
    # rejects higher-rank dram tensors.  The out DMAs use explicit strided
    # APs, so (head, d) runs stay 512B-contiguous (full DMA rate).
    y_out = nc.dram_tensor("y_out", [T, CC], f32, kind="ExternalOutput").ap()

    with tile.TileContext(nc) as tc:
        with (
            tc.tile_pool(name="persist", bufs=1) as pp,
            tc.tile_pool(name="psum", bufs=1, space="PSUM") as psp,
            tc.tile_pool(name="xpool", bufs=2) as xp,
            tc.tile_pool(name="ptpool", bufs=8) as ptp,
            tc.tile_pool(name="opool", bufs=1) as op,
        ):
            # ---- persistent SBUF state ----
            qk_sb = [pp.tile([128, T], bf16, name=f"qk{ct}") for ct in range(8)]
            # v plus a ones column per head: [t-part, head, j-tile, 65]
            v1_sb = pp.tile([128, HPC, NJ, D + 1], bf16, name="v1")
            bqk_sb = pp.tile([128, 8], f32, name="bqk")
            bv_sb = pp.tile([1, CC], f32r, name="bv")
            ones_sb = pp.tile([1, 128], f32r, name="ones")
            wqk_t = []
            wv_t = []

            # softmax-denominator ones column (written once; v copies fill 0:D)
            nc.gpsimd.memset(v1_sb[:, :, :, D : D + 1], 1.0)

            xs_tb = {}

            def load_x(tb):
                tsl = slice(tb * 512, (tb + 1) * 512)
                xs = []
                for e in range(ES):
                    xe = xp.tile([128, 512], f32r, tag=f"x{e}", bufs=2,
                                 name=f"x{e}_{tb}")
                    nc.sync.dma_start(out=xe, in_=xT[e * 128 : (e + 1) * 128, tsl])
                    xs.append(xe)
                xs_tb[tb] = xs

            # small constants first, then x0/wqk interleaved per e-tile (the
            # exp-critical path: pair 0's q/k projection), then wv last (v
            # groups emit after pair 0's q/k in the PE stream anyway)
            nc.sync.dma_start(out=bqk_sb, in_=b_qk)
            nc.sync.dma_start(out=bv_sb, in_=b_v)
            nc.sync.dma_start(out=ones_sb, in_=ones_d)
            tsl0 = slice(0, 512)
            xs0 = []
            # (host packs w_qk cols pr-major: pr*256+[0:128]=q, +[128:256]=k)
            for e in range(ES):
                xe = xp.tile([128, 512], f32r, tag=f"x{e}", bufs=2,
                             name=f"x{e}_0")
                nc.sync.dma_start(out=xe, in_=xT[e * 128 : (e + 1) * 128, tsl0])
                xs0.append(xe)
                wqk = pp.tile([128, 2 * CC], f32r, name=f"wqk{e}")
                nc.sync.dma_start(out=wqk, in_=w_qk[e * 128 : (e + 1) * 128, :])
                wqk_t.append(wqk)
            xs_tb[0] = xs0
            for e in range(ES):
                wv = pp.tile([128, CC], f32r, name=f"wv{e}")
                nc.sync.dma_start(out=wv, in_=w_v[e * 128 : (e + 1) * 128, :])
                wv_t.append(wv)

            def qkv_group_qk(tb, ct, lo=0, hi=ES, cell=None):
                """Emit e-tiles [lo, hi) of the ct projection group; the
                last chunk appends the DVE bias-add.  cell carries the psum
                tile between chunks so groups can be woven in small pieces
                that don't block QK matmuls on the in-order PE queue."""
                tsl = slice(tb * 512, (tb + 1) * 512)
                xs = xs_tb[tb]
                co = (ct % 4) * 256 + (128 if ct >= 4 else 0)
                if cell is None:
                    cell = {}
                if lo == 0:
                    cell["ps"] = psp.tile([128, 512], f32, tag="qp", bufs=2,
                                          name=f"psqk{ct}_{tb}")
                ps = cell["ps"]
                for e in range(lo, hi):
                    nc.tensor.matmul(
                        ps,
                        wqk_t[e][:, co : co + 128],
                        xs[e],
                        start=(e == 0),
                        stop=(e == ES - 1),
                        skip_group_check=True,
                    )
                if hi == ES:
                    # bias add on DVE (psum f32 + [128,1] bias -> sbuf bf16)
                    nc.vector.tensor_scalar_add(
                        qk_sb[ct][:, tsl], ps, bqk_sb[:, ct : ct + 1])

            def qkv_group_v(tb, k4, lo=0, hi=ES, cell=None):
                xs = xs_tb[tb]
                tt = tb * 4 + k4
                if cell is None:
                    cell = {}
                if lo == 0:
                    cell["ps"] = psp.tile([128, 512], f32, tag="qp", bufs=2,
                                          name=f"psv{tt}")
                    nc.tensor.matmul(
                        cell["ps"], ones_sb, bv_sb,
                        start=True, stop=False, skip_group_check=True,
                    )
                psv = cell["ps"]
                for e in range(lo, hi):
                    nc.tensor.matmul(
                        psv,
                        xs[e][:, k4 * 128 : (k4 + 1) * 128],
                        wv_t[e],
                        start=False,
                        stop=(e == ES - 1),
                        skip_group_check=True,
                    )
                if hi == ES:
                    nc.vector.tensor_copy(
                        v1_sb[:, :, tt, 0:D],
                        psv.rearrange("p (h d) -> p h d", d=D),
                    )

            def qk_chunks(tb, cts):
                steps = []
                for ct in cts:
                    cell = {}
                    for lo, hi in ((0, 3), (3, 6), (6, 8)):
                        steps.append(
                            lambda c=ct, l=lo, h=hi, ce=cell:
                            qkv_group_qk(tb, c, l, h, ce))
                return steps

            def v_chunks(tb):
                steps = []
                for g in range(4):
                    cell = {}
                    for lo, hi in ((0, 3), (3, 6), (6, 8)):
                        steps.append(
                            lambda k=g, l=lo, h=hi, ce=cell:
                            qkv_group_v(tb, k, l, h, ce))
                return steps

            def attn_block(I, nxt=(), pre=None):
                nj = 4 * I + 4  # causal j-tiles for this i-block
                yts = {}
                pts = {}

                def qk_exp(pr, J):
                    r = J - 4 * I
                    ws = 128 * r if r > 0 else 0  # causal trim offset
                    qt = qk_sb[pr]
                    kt = qk_sb[4 + pr]
                    jsl = slice(J * 128, (J + 1) * 128)
                    iwl = slice(I * 512 + ws, (I + 1) * 512)
                    st = psp.tile([128, 2, 512], f32, tag="st", bufs=2,
                                  name=f"st{pr}_{I}_{J}")
                    # QK row-tile pair: head A rows 0-63, head B 64-127
                    nc.tensor.matmul(
                        st[:, 0, ws:], kt[0:64, jsl], qt[0:64, iwl],
                        tile_position=(0, 0),
                    )
                    nc.tensor.matmul(
                        st[:, 1, ws:], kt[64:128, jsl], qt[64:128, iwl],
                        tile_position=(64, 0),
                    )
                    pt = ptp.tile([128, 2, 512], bf16, tag="pt",
                                  name=f"pt{pr}_{I}_{J}")
                    nc.scalar.activation(pt[:, :, ws:], st[:, :, ws:],
                                         Act.Exp, scale=0.125)
                    if r >= 0:
                        # causal mask on the 128-wide diagonal band only:
                        # keep where i_band >= j (within-tile coords)
                        nc.gpsimd.affine_select(
                            out=pt[:, :, 128 * r : 128 * (r + 1)],
                            in_=pt[:, :, 128 * r : 128 * (r + 1)],
                            compare_op=is_ge,
                            fill=0.0,
                            base=0,
                            pattern=[[0, 2], [1, 128]],
                            channel_multiplier=-1,
                        )
                    pts[(pr, J)] = pt

                def pv(pr, J):
                    pt = pts.pop((pr, J))
                    ytA, ytB = yts[pr]
                    r = J - 4 * I
                    for h, yt in ((0, ytA), (1, ytB)):
                        for it in range(4):
                            if r > it:
                                continue  # i-tile fully masked for this j
                            # PSUM zeroing is bank-granular: only the FIRST
                            # region of each bank sets start=True; the other
                            # it regions' first writes land on pending-zero
                            # bytes and get a zeroed accumulation base.
                            nc.tensor.matmul(
                                yt[:, it, 0 : D + 1],
                                pt[:, h, it * 128 : (it + 1) * 128],
                                v1_sb[:, 2 * pr + h, J, :],
                                start=(J == 0 and it == 0),
                                stop=(J == 4 * I + it),
                                skip_group_check=True,
                            )

                def out_stage(pr):
                    ytA, ytB = yts.pop(pr)
                    rec = op.tile([128, 2, 4], f32, tag="rec", bufs=2,
                                  name=f"rec{pr}_{I}")
                    yc = op.tile([128, 2, 4, D + 1], f32, tag="yc", bufs=2,
                                 name=f"yc{pr}_{I}")
                    ys = op.tile([128, 4, 2, D], f32, tag="ys", bufs=2,
                                 name=f"ys{pr}_{I}")
                    # copy psum->sbuf on Pool first: releases the yt banks
                    # ~1us earlier so the next pair's PV isn't blocked
                    # behind the normalization reads
                    nc.vector.tensor_copy(yc[:, 0, :, :], ytA[:, :, 0 : D + 1])
                    nc.vector.tensor_copy(yc[:, 1, :, :], ytB[:, :, 0 : D + 1])
                    # denominators live in column 64 of each (h, it) slot
                    nc.vector.reciprocal(rec[:, 0, :], yc[:, 0, :, D])
                    nc.vector.reciprocal(rec[:, 1, :], yc[:, 1, :, D])
                    for h in (0, 1):
                        for it in range(4):
                            nc.vector.tensor_scalar_mul(
                                ys[:, it, h, :], yc[:, h, it, 0:D],
                                rec[:, h, it : it + 1])
                    # ys [i-part, it, h, d] -> y_out[I, it, p, 2pr+h, d];
                    # (head, d) for this pair is 512B contiguous in DRAM
                    import concourse.bass as bass

                    out_ap = bass.AP(
                        tensor=y_out.tensor,
                        offset=I * (4 * 128 * CC) + 2 * pr * D,
                        ap=[[CC, 128], [128 * CC, 4], [1, 2 * D]],
                    )
                    nc.sync.dma_start(
                        out=out_ap,
                        in_=ys.rearrange("p i h d -> p i (h d)"),
                    )

                def alloc_yt(pr):
                    # padded to 4x128 = one full 2KB PSUM bank per head so
                    # the bank-granular start=True zeroing touches no other
                    # tile; regions are [it, 0:65] (64 d cols + denominator)
                    yts[pr] = (
                        psp.tile([128, 4, 128], f32, tag="ytA", bufs=1,
                                 name=f"ytA{pr}_{I}"),
                        psp.tile([128, 4, 128], f32, tag="ytB", bufs=1,
                                 name=f"ytB{pr}_{I}"),
                    )

                items = [(pr, J) for pr in range(PAIRS) for J in range(nj)]
                nxt = list(nxt)
                nsteps = len(nxt)
                popped = 0
                emitted = 0

                def emit_qk(k):
                    pr, J = items[k]
                    if J == 0:
                        if pre:
                            for fn in pre.get(pr, ()):
                                fn()
                        alloc_yt(pr)
                    qk_exp(pr, J)

                for k in range(len(items)):
                    while emitted < min(k + 3, len(items)):
                        emit_qk(emitted)
                        emitted += 1
                    pr, J = items[k]
                    pv(pr, J)
                    if J == nj - 1:
                        out_stage(pr)
                    # weave next t-block's QKV in small chunks so a long
                    # projection burst never delays the next QK (which
                    # would starve ScalarE's exp pipeline)
                    want = (k + 1) * nsteps // len(items)
                    while popped < want:
                        nxt[popped]()
                        popped += 1
                for fn in nxt[popped:]:
                    fn()

            # schedule: per-pair staging for EVERY block.  Block I weaves
            # only what block I+1 needs at its start (v groups + pair 0's
            # q/k); pairs 1-3's q/k groups emit as `pre` inside block I+1,
            # overlapping pair 0's exp backlog.  Prologue: pair 0's q/k
            # first (exp path), then v.
            qkv_group_qk(0, 0)
            qkv_group_qk(0, 4)
            for g in range(4):
                qkv_group_v(0, g)
            pres = {
                0: {
                    pr: qk_chunks(0, [pr, 4 + pr])
                    for pr in range(1, PAIRS)
                }
            }
            for I in range(TB):
                nxt = []
                if I + 1 < TB:
                    load_x(I + 1)
                    nxt = v_chunks(I + 1) + qk_chunks(I + 1, [0, 4])
                    pres[I + 1] = {
                        pr: qk_chunks(I + 1, [pr, 4 + pr])
                        for pr in range(1, PAIRS)
                    }
                attn_block(I, nxt, pre=pres.get(I))
    nc.compile()
    return nc


def get_nc():
    if "nc" not in _cache:
        _cache["nc"] = _build_nc()
    return _cache["nc"]


def shard_inputs(x, w_attn, b_attn):
    """Full inputs -> per-core input maps (host-side slicing/transposition)."""
    x = np.asarray(x, dtype=np.float32)
    w = np.asarray(w_attn, dtype=np.float32)
    bb = np.asarray(b_attn, dtype=np.float32)
    in_maps = []
    for core in range(N_CORES):
        b, hg = core // 2, core % 2
        r0 = hg * CC  # first q row for this head group
        # head-pair-major column packing: pr*256+[0:128]=q(pr), +[128:256]=k(pr)
        wq = w[r0 : r0 + CC, :]
        wk = w[C + r0 : C + r0 + CC, :]
        w_qk = np.ascontiguousarray(
            np.concatenate(
                sum(
                    (
                        [wq[pr * 128 : (pr + 1) * 128], wk[pr * 128 : (pr + 1) * 128]]
                        for pr in range(PAIRS)
                    ),
                    [],
                ),
                axis=0,
            ).T
        )
        w_v = np.ascontiguousarray(w[2 * C + r0 : 2 * C + r0 + CC, :].T)
        b_qk = np.stack(
            [bb[r0 + ct * 128 : r0 + (ct + 1) * 128] for ct in range(4)]
            + [bb[C + r0 + ct * 128 : C + r0 + (ct + 1) * 128] for ct in range(4)],
            axis=1,
        ).astype(np.float32)
        b_v = bb[2 * C + r0 : 2 * C + r0 + CC].reshape(1, CC).astype(np.float32)
        in_maps.append(
            {
                "xT": np.ascontiguousarray(x[b].T),
                "w_qk": w_qk,
                "w_v": w_v,
                "b_qk": np.ascontiguousarray(b_qk),
                "b_v": np.ascontiguousarray(b_v),
                "ones_d": np.ones((1, 128), dtype=np.float32),
            }
        )
    return in_maps


def run(in_maps, trace=False, **kw):
    from concourse import bass_utils

    nc = get_nc()
    return bass_utils.run_bass_kernel_spmd(
        nc, in_maps, core_ids=list(range(N_CORES)), trace=trace, **kw
    )


def gather_output(results):
    y = np.empty((B, T, E), dtype=np.float32)
    for core in range(N_CORES):
        b, hg = core // 2, core % 2
        y[b, :, hg * CC : (hg + 1) * CC] = results[core]["y_out"].reshape(T, CC)
    return y


def kernel(x, w_attn, b_attn):
    in_maps = shard_inputs(x, w_attn, b_attn)
    res = run(in_maps, trace=False)
    return gather_output(res.results)


# revision 35
# speedup vs baseline: 1.0400x; 1.0247x over previous
"""Causal self-attention (B=4, T=2048, E=1024, H=16) on 8 trn2 NeuronCores.

Sharding: core c -> (batch b = c // 2, head-group hg = c % 2); each core owns
one batch element and 8 of the 16 heads (data parallel on B, tensor parallel
on heads).  No cross-core communication.

v2 design (all-bf16 attention, flipped PV):
  - QKV projection in f32r (1 cyc/row at 512 moving); q/k bias-add moved to
    DVE (tensor_scalar_add, psum f32 -> sbuf bf16), freeing ScalarE for exp.
  - QK: bf16 row-tiled head pairs (2 heads per 512-cycle pass), causal
    trim per j-tile (moving width 512-128r on diagonal tiles).
  - exp on ScalarE per j-tile (st psum [128, 2head, 512] -> pt bf16), width
    trimmed like QK; causal mask via gpsimd affine_select on the 128-wide
    diagonal band of pt only.
  - PV flipped: pt is the STATIONARY operand ([j, i-tile] 128 cols), v the
    moving one ([j, d+ones] 65 cols) -> out yt [i-part, 65] costs 65 cycles
    per (head, j-tile, i-tile) vs 512 for the [d, i] orientation.  The
    softmax denominator (ones column of v) lands per-partition, so
    normalization is a [128,8] reciprocal + per-partition-scalar multiplies
    on DVE - no cross-partition broadcast, no DRAM bounce.
  - Output written as y[t, c]; host concatenates without transposing.
"""

import sys

sys.path.insert(0, "/opt/trn_rl_repo")

import numpy as np

N_CORES = 8
B, T, E = 4, 2048, 1024
H, D = 16, 64
C = E                 # q/k/v channel count (4th qkv chunk unused)
HPC = H // 2          # heads per core
CC = HPC * D          # per-core channels = 512
ES = E // 128         # 8 e-tiles (contraction)
TB = T // 512         # 4 t/i blocks of 512
NJ = T // 128         # 16 j-tiles of 128
PAIRS = HPC // 2      # 4 head pairs per core

_cache = {}


def _build_nc():
    import concourse.mybir as mybir
    import concourse.tile as tile
    from concourse import bacc

    f32 = mybir.dt.float32
    f32r = mybir.dt.float32r
    bf16 = mybir.dt.bfloat16
    Act = mybir.ActivationFunctionType
    is_ge = mybir.AluOpType.is_ge

    nc = bacc.Bacc("TRN2", target_bir_lowering=False, debug=False)

    xT = nc.dram_tensor("xT", [E, T], f32r, kind="ExternalInput").ap()
    w_qk = nc.dram_tensor("w_qk", [E, 2 * CC], f32r, kind="ExternalInput").ap()
    w_v = nc.dram_tensor("w_v", [E, CC], f32r, kind="ExternalInput").ap()
    b_qk = nc.dram_tensor("b_qk", [128, 8], f32, kind="ExternalInput").ap()
    b_v = nc.dram_tensor("b_v", [1, CC], f32r, kind="ExternalInput").ap()
    ones_d = nc.dram_tensor("ones_d", [1, 128], f32r, kind="ExternalInput").ap()
    # flat [(I, it, p), (head, d)] == [T, CC] row-major: row I*512+it*128+p
    # is exactly t, cols are c = head*64+d.  Kept 2-D: the PJRT lowering
    # rejects higher-rank dram tensors.  The out DMAs use explicit strided
    # APs, so (head, d) runs stay 512B-contiguous (full DMA rate).
    y_out = nc.dram_tensor("y_out", [T, CC], f32, kind="ExternalOutput").ap()

    with tile.TileContext(nc) as tc:
        with (
            tc.tile_pool(name="persist", bufs=1) as pp,
            tc.tile_pool(name="psum", bufs=1, space="PSUM") as psp,
            tc.tile_pool(name="xpool", bufs=2) as xp,
            tc.tile_pool(name="ptpool", bufs=8) as ptp,
            tc.tile_pool(name="opool", bufs=1) as op,
        ):
            # ---- persistent SBUF state ----
            qk_sb = [pp.tile([128, T], bf16, name=f"qk{ct}") for ct in range(8)]
            # v plus a ones column per head: [t-part, head, j-tile, 65]
            v1_sb = pp.tile([128, HPC, NJ, D + 1], bf16, name="v1")
            bqk_sb = pp.tile([128, 8], f32, name="bqk")
            bv_sb = pp.tile([1, CC], f32r, name="bv")
            ones_sb = pp.tile([1, 128], f32r, name="ones")
            wqk_t = []
            wv_t = []

            # softmax-denominator ones column (written once; v copies fill 0:D)
            nc.gpsimd.memset(v1_sb[:, :, :, D : D + 1], 1.0)

            xs_tb = {}

            def load_x(tb):
                tsl = slice(tb * 512, (tb + 1) * 512)
                xs = []
                for e in range(ES):
                    xe = xp.tile([128, 512], f32r, tag=f"x{e}", bufs=2,
                                 name=f"x{e}_{tb}")
                    nc.sync.dma_start(out=xe, in_=xT[e * 128 : (e + 1) * 128, tsl])
                    xs.append(xe)
                xs_tb[tb] = xs

            # small constants first, then x0/wqk interleaved per e-tile (the
            # exp-critical path: pair 0's q/k projection), then wv last (v
            # groups emit after pair 0's q/k in the PE stream anyway)
            nc.sync.dma_start(out=bqk_sb, in_=b_qk)
            nc.sync.dma_start(out=bv_sb, in_=b_v)
            nc.sync.dma_start(out=ones_sb, in_=ones_d)
            tsl0 = slice(0, 512)
            xs0 = []
            # (host packs w_qk cols pr-major: pr*256+[0:128]=q, +[128:256]=k)
            for e in range(ES):
                xe = xp.tile([128, 512], f32r, tag=f"x{e}", bufs=2,
                             name=f"x{e}_0")
                nc.sync.dma_start(out=xe, in_=xT[e * 128 : (e + 1) * 128, tsl0])
                xs0.append(xe)
                wqk = pp.tile([128, 2 * CC], f32r, name=f"wqk{e}")
                nc.sync.dma_start(out=wqk[:, 0:512],
                                  in_=w_qk[e * 128 : (e + 1) * 128, 0:512])
                wqk_t.append(wqk)
            xs_tb[0] = xs0
            for e in range(ES):
                wv = pp.tile([128, CC], f32r, name=f"wv{e}")
                nc.sync.dma_start(out=wv, in_=w_v[e * 128 : (e + 1) * 128, :])
                wv_t.append(wv)
            for e in range(ES):
                nc.sync.dma_start(out=wqk_t[e][:, 512:1024],
                                  in_=w_qk[e * 128 : (e + 1) * 128, 512:1024])

            def qkv_group_qk(tb, ct, lo=0, hi=ES, cell=None):
                """Emit e-tiles [lo, hi) of the ct projection group; the
                last chunk appends the DVE bias-add.  cell carries the psum
                tile between chunks so groups can be woven in small pieces
                that don't block QK matmuls on the in-order PE queue."""
                tsl = slice(tb * 512, (tb + 1) * 512)
                xs = xs_tb[tb]
                co = (ct % 4) * 256 + (128 if ct >= 4 else 0)
                if cell is None:
                    cell = {}
                if lo == 0:
                    cell["ps"] = psp.tile([128, 512], f32, tag="qp", bufs=2,
                                          name=f"psqk{ct}_{tb}")
                ps = cell["ps"]
                for e in range(lo, hi):
                    nc.tensor.matmul(
                        ps,
                        wqk_t[e][:, co : co + 128],
                        xs[e],
                        start=(e == 0),
                        stop=(e == ES - 1),
                        skip_group_check=True,
                    )
                if hi == ES:
                    # bias add on DVE (psum f32 + [128,1] bias -> sbuf bf16)
                    nc.vector.tensor_scalar_add(
                        qk_sb[ct][:, tsl], ps, bqk_sb[:, ct : ct + 1])

            def qkv_group_v(tb, k4, lo=0, hi=ES, cell=None):
                xs = xs_tb[tb]
                tt = tb * 4 + k4
                if cell is None:
                    cell = {}
                if lo == 0:
                    cell["ps"] = psp.tile([128, 512], f32, tag="qp", bufs=2,
                                          name=f"psv{tt}")
                    nc.tensor.matmul(
                        cell["ps"], ones_sb, bv_sb,
                        start=True, stop=False, skip_group_check=True,
                    )
                psv = cell["ps"]
                for e in range(lo, hi):
                    nc.tensor.matmul(
                        psv,
                        xs[e][:, k4 * 128 : (k4 + 1) * 128],
                        wv_t[e],
                        start=False,
                        stop=(e == ES - 1),
                        skip_group_check=True,
                    )
                if hi == ES:
                    nc.vector.tensor_copy(
                        v1_sb[:, :, tt, 0:D],
                        psv.rearrange("p (h d) -> p h d", d=D),
                    )

            def qk_chunks(tb, cts):
                steps = []
                for ct in cts:
                    cell = {}
                    for lo, hi in ((0, 3), (3, 6), (6, 8)):
                        steps.append(
                            lambda c=ct, l=lo, h=hi, ce=cell:
                            qkv_group_qk(tb, c, l, h, ce))
                return steps

            def v_chunks(tb):
                steps = []
                for g in range(4):
                    cell = {}
                    for lo, hi in ((0, 3), (3, 6), (6, 8)):
                        steps.append(
                            lambda k=g, l=lo, h=hi, ce=cell:
                            qkv_group_v(tb, k, l, h, ce))
                return steps

            def attn_block(I, nxt=(), pre=None):
                nj = 4 * I + 4  # causal j-tiles for this i-block
                yts = {}
                pts = {}

                def qk_exp(pr, J):
                    r = J - 4 * I
                    ws = 128 * r if r > 0 else 0  # causal trim offset
                    qt = qk_sb[pr]
                    kt = qk_sb[4 + pr]
                    jsl = slice(J * 128, (J + 1) * 128)
                    iwl = slice(I * 512 + ws, (I + 1) * 512)
                    st = psp.tile([128, 2, 512], f32, tag="st", bufs=2,
                                  name=f"st{pr}_{I}_{J}")
                    # QK row-tile pair: head A rows 0-63, head B 64-127
                    nc.tensor.matmul(
                        st[:, 0, ws:], kt[0:64, jsl], qt[0:64, iwl],
                        tile_position=(0, 0),
                    )
                    nc.tensor.matmul(
                        st[:, 1, ws:], kt[64:128, jsl], qt[64:128, iwl],
                        tile_position=(64, 0),
                    )
                    pt = ptp.tile([128, 2, 512], bf16, tag="pt",
                                  name=f"pt{pr}_{I}_{J}")
                    nc.scalar.activation(pt[:, :, ws:], st[:, :, ws:],
                                         Act.Exp, scale=0.125)
                    if r >= 0:
                        # causal mask on the 128-wide diagonal band only:
                        # keep where i_band >= j (within-tile coords)
                        nc.gpsimd.affine_select(
                            out=pt[:, :, 128 * r : 128 * (r + 1)],
                            in_=pt[:, :, 128 * r : 128 * (r + 1)],
                            compare_op=is_ge,
                            fill=0.0,
                            base=0,
                            pattern=[[0, 2], [1, 128]],
                            channel_multiplier=-1,
                        )
                    pts[(pr, J)] = pt

                def pv(pr, J):
                    pt = pts.pop((pr, J))
                    ytA, ytB = yts[pr]
                    r = J - 4 * I
                    for h, yt in ((0, ytA), (1, ytB)):
                        for it in range(4):
                            if r > it:
                                continue  # i-tile fully masked for this j
                            # PSUM zeroing is bank-granular: only the FIRST
                            # region of each bank sets start=True; the other
                            # it regions' first writes land on pending-zero
                            # bytes and get a zeroed accumulation base.
                            nc.tensor.matmul(
                                yt[:, it, 0 : D + 1],
                                pt[:, h, it * 128 : (it + 1) * 128],
                                v1_sb[:, 2 * pr + h, J, :],
                                start=(J == 0 and it == 0),
                                stop=(J == 4 * I + it),
                                skip_group_check=True,
                            )

                def out_stage(pr):
                    ytA, ytB = yts.pop(pr)
                    rec = op.tile([128, 2, 4], f32, tag="rec", bufs=2,
                                  name=f"rec{pr}_{I}")
                    yc = op.tile([128, 2, 4, D + 1], f32, tag="yc", bufs=2,
                                 name=f"yc{pr}_{I}")
                    ys = op.tile([128, 4, 2, D], f32, tag="ys", bufs=2,
                                 name=f"ys{pr}_{I}")
                    # copy psum->sbuf on Pool first: releases the yt banks
                    # ~1us earlier so the next pair's PV isn't blocked
                    # behind the normalization reads
                    nc.vector.tensor_copy(yc[:, 0, :, :], ytA[:, :, 0 : D + 1])
                    nc.vector.tensor_copy(yc[:, 1, :, :], ytB[:, :, 0 : D + 1])
                    # denominators live in column 64 of each (h, it) slot
                    nc.vector.reciprocal(rec[:, 0, :], yc[:, 0, :, D])
                    nc.vector.reciprocal(rec[:, 1, :], yc[:, 1, :, D])
                    for h in (0, 1):
                        for it in range(4):
                            nc.vector.tensor_scalar_mul(
                                ys[:, it, h, :], yc[:, h, it, 0:D],
                                rec[:, h, it : it + 1])
                    # ys [i-part, it, h, d] -> y_out[I, it, p, 2pr+h, d];
                    # (head, d) for this pair is 512B contiguous in DRAM
                    import concourse.bass as bass

                    out_ap = bass.AP(
                        tensor=y_out.tensor,
                        offset=I * (4 * 128 * CC) + 2 * pr * D,
                        ap=[[CC, 128], [128 * CC, 4], [1, 2 * D]],
                    )
                    nc.sync.dma_start(
                        out=out_ap,
                        in_=ys.rearrange("p i h d -> p i (h d)"),
                    )

                def alloc_yt(pr):
                    # padded to 4x128 = one full 2KB PSUM bank per head so
                    # the bank-granular start=True zeroing touches no other
                    # tile; regions are [it, 0:65] (64 d cols + denominator)
                    yts[pr] = (
                        psp.tile([128, 4, 128], f32, tag="ytA", bufs=1,
                                 name=f"ytA{pr}_{I}"),
                        psp.tile([128, 4, 128], f32, tag="ytB", bufs=1,
                                 name=f"ytB{pr}_{I}"),
                    )

                items = [(pr, J) for pr in range(PAIRS) for J in range(nj)]
                nxt = list(nxt)
                nsteps = len(nxt)
                popped = 0
                emitted = 0

                def emit_qk(k):
                    pr, J = items[k]
                    if J == 0:
                        if pre:
                            for fn in pre.get(pr, ()):
                                fn()
                        alloc_yt(pr)
                    qk_exp(pr, J)

                for k in range(len(items)):
                    while emitted < min(k + 4, len(items)):
                        emit_qk(emitted)
                        emitted += 1
                    pr, J = items[k]
                    pv(pr, J)
                    if J == nj - 1:
                        out_stage(pr)
                    # weave next t-block's QKV in small chunks so a long
                    # projection burst never delays the next QK (which
                    # would starve ScalarE's exp pipeline)
                    want = (k + 1) * nsteps // len(items)
                    while popped < want:
                        nxt[popped]()
                        popped += 1
                for fn in nxt[popped:]:
                    fn()

            # schedule: per-pair staging for EVERY block.  Block I weaves
            # only what block I+1 needs at its start (v groups + pair 0's
            # q/k); pairs 1-3's q/k groups emit as `pre` inside block I+1,
            # overlapping pair 0's exp backlog.  Prologue: pair 0's q/k
            # first (exp path), then v.
            qkv_group_qk(0, 0)
            qkv_group_qk(0, 4)
            for g in range(4):
                qkv_group_v(0, g)
            pres = {
                0: {
                    pr: qk_chunks(0, [pr, 4 + pr])
                    for pr in range(1, PAIRS)
                }
            }
            for I in range(TB):
                nxt = []
                if I + 1 < TB:
                    load_x(I + 1)
                    nxt = v_chunks(I + 1) + qk_chunks(I + 1, [0, 4])
                    pres[I + 1] = {
                        pr: qk_chunks(I + 1, [pr, 4 + pr])
                        for pr in range(1, PAIRS)
                    }
                attn_block(I, nxt, pre=pres.get(I))
    nc.compile()
    return nc


def get_nc():
    if "nc" not in _cache:
        _cache["nc"] = _build_nc()
    return _cache["nc"]


def shard_inputs(x, w_attn, b_attn):
    """Full inputs -> per-core input maps (host-side slicing/transposition)."""
    x = np.asarray(x, dtype=np.float32)
    w = np.asarray(w_attn, dtype=np.float32)
    bb = np.asarray(b_attn, dtype=np.float32)
    in_maps = []
    for core in range(N_CORES):
        b, hg = core // 2, core % 2
        r0 = hg * CC  # first q row for this head group
        # head-pair-major column packing: pr*256+[0:128]=q(pr), +[128:256]=k(pr)
        wq = w[r0 : r0 + CC, :]
        wk = w[C + r0 : C + r0 + CC, :]
        w_qk = np.ascontiguousarray(
            np.concatenate(
                sum(
                    (
                        [wq[pr * 128 : (pr + 1) * 128], wk[pr * 128 : (pr + 1) * 128]]
                        for pr in range(PAIRS)
                    ),
                    [],
                ),
                axis=0,
            ).T
        )
        w_v = np.ascontiguousarray(w[2 * C + r0 : 2 * C + r0 + CC, :].T)
        b_qk = np.stack(
            [bb[r0 + ct * 128 : r0 + (ct + 1) * 128] for ct in range(4)]
            + [bb[C + r0 + ct * 128 : C + r0 + (ct + 1) * 128] for ct in range(4)],
            axis=1,
        ).astype(np.float32)
        b_v = bb[2 * C + r0 : 2 * C + r0 + CC].reshape(1, CC).astype(np.float32)
        in_maps.append(
            {
                "xT": np.ascontiguousarray(x[b].T),
                "w_qk": w_qk,
                "w_v": w_v,
                "b_qk": np.ascontiguousarray(b_qk),
                "b_v": np.ascontiguousarray(b_v),
                "ones_d": np.ones((1, 128), dtype=np.float32),
            }
        )
    return in_maps


def run(in_maps, trace=False, **kw):
    from concourse import bass_utils

    nc = get_nc()
    return bass_utils.run_bass_kernel_spmd(
        nc, in_maps, core_ids=list(range(N_CORES)), trace=trace, **kw
    )


def gather_output(results):
    y = np.empty((B, T, E), dtype=np.float32)
    for core in range(N_CORES):
        b, hg = core // 2, core % 2
        y[b, :, hg * CC : (hg + 1) * CC] = results[core]["y_out"].reshape(T, CC)
    return y


def kernel(x, w_attn, b_attn):
    in_maps = shard_inputs(x, w_attn, b_attn)
    res = run(in_maps, trace=False)
    return gather_output(res.results)
